# revision 1
# baseline (speedup 1.0000x reference)
"""CavityLoss Trainium2 kernel (nn_CavityLoss_43722767073667).

Mathematical reduction of the reference, exact in fp32 (verified against a
bit-faithful numpy emulation incl. adversarial threshold-boundary values):

  pb = (floor(pred*255) >= 128)  <=>  (pred >= c*),  c* = f32(128/255)
  The 5^3 all-ones dilation of the binary gt is an exact integer count
  >= gt (the window contains the center voxel), so
      diff = ((gt - pb*dilate(gt)) > 0) == gt * (1 - pb)     [identity]
  Non-critical voxels contribute exactly 0 to the BCE in fp32:
      clip(0, 1e-12, 1-1e-12) -> 1e-12, and fp32(1 - 1e-12) == 1.0,
      so (1-lc)*log(1-pc_c) == log(1.0) == 0.
  Therefore  loss = -mean( gt * [pred < c*] * ln(pred) ).

Distribution: 192^3 volume flattened and split into 8 equal slabs (depth
sharding: 24 z-planes per core), each viewed as [128 partitions, 6912].
Pointwise + reduction only - the dilation cancels, so no halo exchange and
no collectives; the cross-core mean is combined on the host in f64.

Per-core device kernel (raw bacc, hand-rolled semaphores, no Tile):
  sync engine streams pred/gt tiles in on the qSP HWDGE ring
  DVE   STT#1: r = (p is_ge c*) max p        # r = p where p<c*, else 1.0
  ACT   Ln:    l = ln(r)                     # masked ln; ln(1) ~ 0
  DVE   STT#2: (l bypass 1) mult gt, accum_out -> per-partition row sums
  PE    ones^T @ acc                         # 128-partition reduce -> [1,NT]
  sync  one contiguous 20-byte DMA of the [1,NT] result

Scheduling notes (measured on HW):
  - one semaphore per DMA transfer (completion order across queues is not
    FIFO, a shared counter would race - caught by CoreSim)
  - exactly one wait per instruction (TRN2 HW limit; gt arrival is proxied
    through ACT's wait so DVE's STT#2 only waits on s_l)
  - DVE stream is software-pipelined (STT#1(t+1) before STT#2(t)) so the
    serial STT#1->Ln->STT#2 chain spans tiles instead of serializing
  - a dummy Ln on the const-1.0 tile hoists the ~2.7us ACT_TABLE_LOAD
    into the DMA wait window
  - progressive tile sizes: the last tile is small so the post-last-byte
    compute tail (Ln + STT#2 of the final tile) is short
"""

import numpy as np

import concourse.bacc as bacc
import concourse.mybir as mybir
from concourse.bass_utils import run_bass_kernel_spmd

D = 192
N_CORES = 8
P = 128
TOTAL = D * D * D              # 7_077_888
PER_CORE = TOTAL // N_CORES    # 884_736
FREE = PER_CORE // P           # 6_912
SIZES = [1728, 1728, 1728, 1152, 576]
assert sum(SIZES) == FREE
NT = len(SIZES)

C_STAR = float(np.float32(128.0) / np.float32(255.0))

_CACHE = {}


def _build():
    nc = bacc.Bacc("TRN2", name="cavity_loss")
    f32 = mybir.dt.float32
    pred = nc.dram_tensor("pred", [P, FREE], f32, kind="ExternalInput")
    gt = nc.dram_tensor("gt", [P, FREE], f32, kind="ExternalInput")
    out = nc.dram_tensor("out", [1, NT], f32, kind="ExternalOutput")

    ge = mybir.AluOpType.is_ge
    mx = mybir.AluOpType.max
    byp = mybir.AluOpType.bypass
    mul = mybir.AluOpType.mult
    Ln = mybir.ActivationFunctionType.Ln

    pred_sb = nc.alloc_sbuf_tensor("pred_sb", [P, FREE], f32).ap()
    gt_sb = nc.alloc_sbuf_tensor("gt_sb", [P, FREE], f32).ap()
    r_sb = nc.alloc_sbuf_tensor("r_sb", [P, FREE], f32).ap()
    l_sb = nc.alloc_sbuf_tensor("l_sb", [P, FREE], f32).ap()
    acc = nc.alloc_sbuf_tensor("acc_sb", [P, NT], f32).ap()

    s_pred = [nc.alloc_semaphore(f"s_pred{t}") for t in range(NT)]
    s_gt = [nc.alloc_semaphore(f"s_gt{t}") for t in range(NT)]
    s_r = nc.alloc_semaphore("s_r")
    s_l = nc.alloc_semaphore("s_l")
    s_acc = nc.alloc_semaphore("s_acc")
    s_mm = nc.alloc_semaphore("s_mm")
    s_fin = nc.alloc_semaphore("s_fin")
    s_out = nc.alloc_semaphore("s_out")

    offs = np.concatenate([[0], np.cumsum(SIZES)]).tolist()
    sls = [slice(offs[t], offs[t + 1]) for t in range(NT)]

    # sync: stream all tiles in on one HWDGE ring, pred before gt per tile
    for t in range(NT):
        nc.sync.dma_start(pred_sb[:, sls[t]], pred[:, sls[t]]).then_inc(s_pred[t], 16)
        nc.sync.dma_start(gt_sb[:, sls[t]], gt[:, sls[t]]).then_inc(s_gt[t], 16)

    # scalar: dummy Ln pulls ACT_TABLE_LOAD into the DMA window, then the
    # per-tile Ln chain (gt arrival proxied so STT#2 needs a single wait)
    dummy = nc.alloc_sbuf_tensor("dummy_sb", [P, 1], f32).ap()
    nc.scalar.activation(dummy[:], nc.const_aps.tensor(1.0, (P, 1)), Ln)
    for t in range(NT):
        sl = sls[t]
        nc.scalar.wait_ge(s_gt[t], 16)
        nc.scalar.wait_ge(s_r, t + 1)
        nc.scalar.activation(l_sb[:, sl], r_sb[:, sl], Ln).then_inc(s_l, 1)

    # vector, software-pipelined across tiles
    def stt1(t):
        sl = sls[t]
        nc.vector.wait_ge(s_pred[t], 16)
        nc.vector.scalar_tensor_tensor(
            r_sb[:, sl], pred_sb[:, sl], C_STAR, pred_sb[:, sl], ge, mx
        ).then_inc(s_r, 1)

    def stt2(t):
        sl = sls[t]
        nc.vector.wait_ge(s_l, t + 1)
        # out lands over r_sb tile t: dead after Ln(t), ordered via s_l wait
        nc.vector.scalar_tensor_tensor(
            r_sb[:, sl], l_sb[:, sl], 1.0, gt_sb[:, sl], byp, mul,
            accum_out=acc[:, t : t + 1],
        ).then_inc(s_acc, 1)

    stt1(0)
    for t in range(1, NT):
        stt1(t)
        stt2(t - 1)
    stt2(NT - 1)

    # finalize: partition-reduce acc on the (otherwise idle) TensorEngine,
    # then one contiguous tiny DMA: [1, NT] on one partition = 1 descriptor
    psum_fin = nc.alloc_psum_tensor("psum_fin", [1, NT], f32).ap()
    fin_sb = nc.alloc_sbuf_tensor("fin_sb", [1, NT], f32).ap()
    ones = nc.const_aps.tensor(1.0, (P, 1))
    nc.tensor.wait_ge(s_acc, NT)
    nc.tensor.matmul(
        psum_fin[:], ones, acc[:], start=True, stop=True
    ).then_inc(s_mm, 1)
    nc.vector.wait_ge(s_mm, 1)
    nc.vector.tensor_copy(fin_sb[:], psum_fin[:]).then_inc(s_fin, 1)
    nc.sync.wait_ge(s_fin, 1)
    nc.sync.dma_start(out[:], fin_sb[:]).then_inc(s_out, 16)
    nc.sync.wait_ge(s_out, 16)

    nc.compile()
    return nc


def _get_nc():
    if "nc" not in _CACHE:
        _CACHE["nc"] = _build()
    return _CACHE["nc"]


def _shard(x):
    flat = np.ascontiguousarray(np.asarray(x, dtype=np.float32)).reshape(-1)
    assert flat.size == TOTAL, f"expected {TOTAL} elements, got {flat.size}"
    return [
        flat[c * PER_CORE : (c + 1) * PER_CORE].reshape(P, FREE)
        for c in range(N_CORES)
    ]


def run_spmd(pred, gt, **kw):
    """Shard, run on 8 cores; returns BassKernelResults (kw e.g. trace=True)."""
    preds = _shard(pred)
    gts = _shard(gt)
    in_maps = [{"pred": preds[c], "gt": gts[c]} for c in range(N_CORES)]
    return run_bass_kernel_spmd(
        _get_nc(), in_maps, core_ids=list(range(N_CORES)), **kw
    )


def kernel(pred, gt):
    res = run_spmd(pred, gt)
    total = 0.0
    for r in res.results:
        total += float(r["out"].astype(np.float64).sum())
    return np.asarray(np.float32(-total / TOTAL))



# revision 2
# speedup vs baseline: 1.2099x; 1.2099x over previous
"""CavityLoss Trainium2 kernel (nn_CavityLoss_43722767073667).

Mathematical reduction of the reference, exact in fp32 (verified against a
bit-faithful numpy emulation incl. adversarial threshold-boundary values):

  pb = (floor(pred*255) >= 128)  <=>  (pred >= c*),  c* = f32(128/255)
  The 5^3 all-ones dilation of the binary gt is an exact integer count
  >= gt (the window contains the center voxel), so
      diff = ((gt - pb*dilate(gt)) > 0) == gt * (1 - pb)     [identity]
  Non-critical voxels contribute exactly 0 to the BCE in fp32:
      clip(0, 1e-12, 1-1e-12) -> 1e-12, and fp32(1 - 1e-12) == 1.0,
      so (1-lc)*log(1-pc_c) == log(1.0) == 0.
  Therefore  loss = -mean( gt * [pred < c*] * ln(pred) ).

Distribution: 192^3 volume flattened and split into 8 equal slabs (depth
sharding: 24 z-planes per core), each viewed as [128 partitions, 6912].
Pointwise + reduction only - the dilation cancels, so no halo exchange and
no collectives; the cross-core mean is combined on the host in f64.

v2 layout (vs the 35.2us v1): the host packs, per tile t, pred and gt
side-by-side into ONE contiguous DRAM tensor in_t = [128, 2*s_t]
(cols [0,s) = pred, [s,2s) = gt).  This halves the transfer count (6 big
fully-contiguous DMAs instead of 10 strided ones), halves the DMA
semaphores/waits, and makes every descriptor sequential in HBM.  Tile
sizes are progressive at both ends: small first tile so the DVE starts
~2us earlier, tiny last tile so the post-last-byte tail (STT#1 -> Ln ->
STT#2) is short.

Per-core device kernel (raw bacc, hand-rolled semaphores, no Tile):
  sync  streams in_t tiles on the qSP HWDGE ring
  DVE   STT#1: r = (p is_ge c*) max p        # r = p where p<c*, else 1.0
  ACT   Ln:    l = ln(r)                     # masked ln; ln(1) ~ 0
  DVE   STT#2: (l bypass 1) mult gt, accum_out -> acc[:, t]
  sync  one DMA of the [128, NT] per-tile partition sums; host reduces
        (no TensorEngine / PSUM finalize - the matmul chain was ~0.5us
        of pure tail)

Scheduling notes (measured on HW):
  - one semaphore per DMA transfer; exactly one wait per instruction
    (TRN2 HW limit).  gt arrival needs no extra proxy wait anymore: the
    merged transfer's semaphore covers both halves via STT#1's wait.
  - DVE stream is software-pipelined (STT#1(t+1) before STT#2(t))
  - a dummy Ln on the const-1.0 tile hoists the ~1.3us ACT_TABLE_LOAD
    into the DMA wait window
"""

import numpy as np

import concourse.bacc as bacc
import concourse.mybir as mybir
from concourse.bass_utils import run_bass_kernel_spmd

D = 192
N_CORES = 8
P = 128
TOTAL = D * D * D              # 7_077_888
PER_CORE = TOTAL // N_CORES    # 884_736
FREE = PER_CORE // P           # 6_912
SIZES = [576, 1152, 2304, 1728, 1024, 128]
assert sum(SIZES) == FREE
NT = len(SIZES)

C_STAR = float(np.float32(128.0) / np.float32(255.0))

_CACHE = {}


def _build():
    nc = bacc.Bacc("TRN2", name="cavity_loss")
    f32 = mybir.dt.float32

    ins = [
        nc.dram_tensor(f"in{t}", [P, 2 * s], f32, kind="ExternalInput")
        for t, s in enumerate(SIZES)
    ]
    out = nc.dram_tensor("out", [P, NT], f32, kind="ExternalOutput")

    ge = mybir.AluOpType.is_ge
    mx = mybir.AluOpType.max
    byp = mybir.AluOpType.bypass
    mul = mybir.AluOpType.mult
    Ln = mybir.ActivationFunctionType.Ln

    in_sb = [
        nc.alloc_sbuf_tensor(f"in_sb{t}", [P, 2 * s], f32).ap()
        for t, s in enumerate(SIZES)
    ]
    r_sb = nc.alloc_sbuf_tensor("r_sb", [P, FREE], f32).ap()
    l_sb = nc.alloc_sbuf_tensor("l_sb", [P, FREE], f32).ap()
    acc = nc.alloc_sbuf_tensor("acc_sb", [P, NT], f32).ap()

    s_in = [nc.alloc_semaphore(f"s_in{t}") for t in range(NT)]
    s_r = nc.alloc_semaphore("s_r")
    s_l = nc.alloc_semaphore("s_l")
    s_acc = nc.alloc_semaphore("s_acc")
    s_out = nc.alloc_semaphore("s_out")

    offs = np.concatenate([[0], np.cumsum(SIZES)]).tolist()
    sls = [slice(offs[t], offs[t + 1]) for t in range(NT)]

    # sync: stream all merged tiles in on one HWDGE ring
    for t in range(NT):
        nc.sync.dma_start(in_sb[t][:, :], ins[t][:, :]).then_inc(s_in[t], 16)

    # scalar: dummy Ln pulls ACT_TABLE_LOAD into the DMA window, then the
    # per-tile Ln chain (single wait: r(t) ready implies in(t) arrived)
    dummy = nc.alloc_sbuf_tensor("dummy_sb", [P, 1], f32).ap()
    nc.scalar.activation(dummy[:], nc.const_aps.tensor(1.0, (P, 1)), Ln)
    for t in range(NT):
        nc.scalar.wait_ge(s_r, t + 1)
        nc.scalar.activation(l_sb[:, sls[t]], r_sb[:, sls[t]], Ln).then_inc(s_l, 1)

    # vector, software-pipelined across tiles
    def stt1(t):
        s = SIZES[t]
        nc.vector.wait_ge(s_in[t], 16)
        nc.vector.scalar_tensor_tensor(
            r_sb[:, sls[t]], in_sb[t][:, 0:s], C_STAR, in_sb[t][:, 0:s], ge, mx
        ).then_inc(s_r, 1)

    def stt2(t):
        s = SIZES[t]
        nc.vector.wait_ge(s_l, t + 1)
        # out lands over r_sb tile t: dead after Ln(t), ordered via s_l wait
        nc.vector.scalar_tensor_tensor(
            r_sb[:, sls[t]], l_sb[:, sls[t]], 1.0, in_sb[t][:, s : 2 * s],
            byp, mul,
            accum_out=acc[:, t : t + 1],
        ).then_inc(s_acc, 1)

    stt1(0)
    for t in range(1, NT):
        stt1(t)
        stt2(t - 1)
    stt2(NT - 1)

    # finalize: one DMA of the [128, NT] partition sums; host reduces
    nc.sync.wait_ge(s_acc, NT)
    nc.sync.dma_start(out[:], acc[:]).then_inc(s_out, 16)
    nc.sync.wait_ge(s_out, 16)

    nc.compile()
    return nc


def _get_nc():
    if "nc" not in _CACHE:
        _CACHE["nc"] = _build()
    return _CACHE["nc"]


_OFFS = np.concatenate([[0], np.cumsum(SIZES)]).tolist()


def _shard(pred, gt):
    """Per core, per tile: one contiguous [128, 2*s] array [pred | gt]."""
    pf = np.ascontiguousarray(np.asarray(pred, dtype=np.float32)).reshape(-1)
    gf = np.ascontiguousarray(np.asarray(gt, dtype=np.float32)).reshape(-1)
    assert pf.size == TOTAL and gf.size == TOTAL
    in_maps = []
    for c in range(N_CORES):
        pc = pf[c * PER_CORE : (c + 1) * PER_CORE].reshape(P, FREE)
        gc = gf[c * PER_CORE : (c + 1) * PER_CORE].reshape(P, FREE)
        m = {}
        for t in range(NT):
            sl = slice(_OFFS[t], _OFFS[t + 1])
            m[f"in{t}"] = np.ascontiguousarray(
                np.concatenate([pc[:, sl], gc[:, sl]], axis=1)
            )
        in_maps.append(m)
    return in_maps


def run_spmd(pred, gt, **kw):
    """Shard, run on 8 cores; returns BassKernelResults (kw e.g. trace=True)."""
    in_maps = _shard(pred, gt)
    return run_bass_kernel_spmd(
        _get_nc(), in_maps, core_ids=list(range(N_CORES)), **kw
    )


def kernel(pred, gt):
    res = run_spmd(pred, gt)
    total = 0.0
    for r in res.results:
        total += float(r["out"].astype(np.float64).sum())
    return np.asarray(np.float32(-total / TOTAL))
